# revision 19
# baseline (speedup 1.0000x reference)
"""Trainium2 Bass kernel for nn_DataPreprocessor: row-interleave + 16x16 patch
extraction, implemented as a pure data-movement (permutation) kernel.

Reference semantics (per sample):
  data: [2, 65536] -> R: [256, 512] with R[2k]=data[0].reshape(128,512)[k],
  R[2k+1]=data[1].reshape(128,512)[k] -> non-overlapping 16x16 patches,
  row-major, each flattened -> out: [512, 256].

Index algebra (per sample), with z1 in [0,16), z2 in [0,32), ph in [0,8),
e in [0,2), q in [0,16):
  out[z1*32+z2, (2*ph+e)*16+q] = data[e, z1*4096 + ph*512 + z2*16 + q]
With out flat = z1*8192 + z2*256 + ph*32 + e*16 + q the kernel is the pure
5D transpose (e, z1, ph, z2, q) -> (z1, z2, ph, e, q).

Strategy: batch-shard 256 samples over 8 cores (32/core), processed as 2
tiles of 16 samples. Split z1 = z1h*2 + z1l; SBUF partition p = b*8 + z1h
(b in [0,16) local). Then:
  - loads (one per e): HBM AP [b:16][z1h:8][(z1l r):8192] -- 32KB contiguous
    descriptors, outer dim 16 so the HWDGE spreads over all 16 SDMA engines
  - shuffle: 4 DVE copies per tile, (e,z1l)-indexed, permuting the free dim
    (ph,z2,q) -> (z2,ph,q) blocks into out order within each partition
  - stores (one per z1l): HBM AP [b:16][z1h:8][8192] -- 32KB descriptors,
    outer 16
All DMA descriptors are the full-rate 32KB shape (the 181us -> 117us -> now
journey: never give HWDGE an AP whose outer dim is <16, and never let
descriptors shrink below 32KB when avoidable).

Raw bass (not Tile): walrus's DMA_DIRECT2D struct admits only one sync-wait
command per DMA, so cross-engine ordering uses standalone wait_ge
instructions. DMA-completion semaphores arrive as 16 independent +1s per
DMA, so each wait threshold must only ever count DMAs covered by it:
dedicated sems per (tile, e) for loads and per tout-slot for stores.
"""

import sys

for _p in ("/opt/trn_rl_repo",):
    if _p not in sys.path:
        sys.path.insert(0, _p)

import numpy as np

import concourse.bass as bass
import concourse.mybir as mybir
from concourse.bass_utils import run_bass_kernel_spmd

N_CORES = 8
B = 256
B_PER_CORE = B // N_CORES          # 32
SAMPLES_PER_TILE = 16              # 16 samples x 8 z1h = 128 partitions
Z1H, Z1L, Z2, PH, E, QQ = 8, 2, 32, 8, 2, 16
FREE_IN = E * Z1L * PH * Z2 * QQ   # 16384 elements = 64KB per partition
FREE_OUT = PH * Z2 * E * QQ        # 8192 elements = 32KB per partition
NPART = 128


def build_nc(b_per_core: int = B_PER_CORE) -> bass.Bass:
    n_tiles = b_per_core // SAMPLES_PER_TILE
    f32 = mybir.dt.float32

    nc = bass.Bass()
    x = nc.dram_tensor("x", [b_per_core, 2, 65536], f32, kind="ExternalInput")
    y = nc.dram_tensor("y", [b_per_core, 512, 256], f32,
                       kind="ExternalOutput")

    # load view: [b, z1h, e, z1l, r] ; r is a 16KB contiguous run
    xv = x.rearrange("b e (z1h z1l r) -> b z1h e z1l r", z1h=Z1H, z1l=Z1L)
    # store view: [b, z1h, z1l, (z2 c)] ; (z2 c) is a 32KB contiguous run
    yv = y.rearrange("b (z1h z1l z2) c -> b z1h z1l (z2 c)",
                     z1h=Z1H, z1l=Z1L)

    with (
        nc.sbuf_tensor([NPART, FREE_IN], f32) as tin0,
        nc.sbuf_tensor([NPART, FREE_IN], f32) as tin1,
        nc.sbuf_tensor([NPART, FREE_OUT], f32) as tout0,
        nc.sbuf_tensor([NPART, FREE_OUT], f32) as tout1,
        nc.semaphore("ld00") as ld00,
        nc.semaphore("ld01") as ld01,
        nc.semaphore("ld10") as ld10,
        nc.semaphore("ld11") as ld11,
        nc.semaphore("st0") as st0,
        nc.semaphore("st1") as st1,
        nc.semaphore("cp_sem") as cp_sem,
        nc.semaphore("cp_gp_sem") as cp_gp_sem,
        nc.Block() as block,
    ):
        tins = [tin0, tin1]
        touts = [tout0, tout1]
        # one sem per (tile, z1l) load-quarter-pair: copy (t,z1l,e) needs
        # only the (e,z1l) quarter, so keying sems by z1l lets the first
        # store start after 2 quarter-loads instead of 4
        ld_sems = [[ld00, ld01], [ld10, ld11]]
        st_sems = [st0, st1]

        @block.sync
        def _(sync):
            # loads stream back-to-back with no waits: each tile has its
            # own tin buffer, so there is no SBUF reuse hazard on loads.
            # One DMA per (e, z1l) quarter: 16KB descriptors -- measured
            # faster on the HBM read side than 32KB descriptors that comb
            # over the e-interleave at 50% duty. e-major issue order keeps
            # consecutive DMAs reading adjacent HBM regions.
            for t in range(n_tiles):
                b0 = t * SAMPLES_PER_TILE
                for e in range(E):
                    for z1l in range(Z1L):
                        off = e * 8192 + z1l * 4096
                        sync.dma_start(
                            out=tins[t][:, off:off + 4096],
                            in_=xv[b0:b0 + SAMPLES_PER_TILE, :, e, z1l],
                        ).then_inc(ld_sems[t][z1l], 16)

        def shuffle_aps(t, z1l, e, tout):
            # src: f = e*8192 + z1l*4096 + ph*512 + z2*16 + q
            src = tins[t].rearrange(
                "p (e z1l ph z2 q) -> p e z1l ph z2 q",
                e=E, z1l=Z1L, ph=PH, z2=Z2, q=QQ)[:, e, z1l]
            # dst: f' = z2*256 + ph*32 + e*16 + q
            dst = tout.rearrange(
                "p (z2 ph e q) -> p e ph z2 q",
                z2=Z2, ph=PH, e=E, q=QQ)[:, e]
            return dst, src

        # the e=0/e=1 halves of each (tile, z1l) shuffle run concurrently
        # on DVE and GpSimd (1-input GpSimd copies run near line rate and
        # Q7 is otherwise idle: all DMAs here are HWDGE). Separate sems
        # per engine: a shared counter would let later DVE increments
        # satisfy a threshold while the GpSimd half still runs.

        @block.vector
        def _(vector):
            for t in range(n_tiles):
                for z1l in range(Z1L):
                    s = (t * Z1L + z1l) % 2
                    if t * Z1L + z1l >= 2:
                        # WAR: the store that last read this tout slot
                        vector.wait_ge(st_sems[s], 16 * ((t * Z1L + z1l) // 2))
                    # both e quarter-loads of this (tile, z1l)
                    vector.wait_ge(ld_sems[t][z1l], 32)
                    dst, src = shuffle_aps(t, z1l, 0, touts[s])
                    vector.tensor_copy(dst, src).then_inc(cp_sem, 1)

        @block.gpsimd
        def _(gpsimd):
            for t in range(n_tiles):
                for z1l in range(Z1L):
                    s = (t * Z1L + z1l) % 2
                    if t * Z1L + z1l >= 2:
                        gpsimd.wait_ge(st_sems[s], 16 * ((t * Z1L + z1l) // 2))
                    gpsimd.wait_ge(ld_sems[t][z1l], 32)
                    dst, src = shuffle_aps(t, z1l, 1, touts[s])
                    gpsimd.tensor_copy(dst, src).then_inc(cp_gp_sem, 1)

        @block.scalar
        def _(scalar):
            for t in range(n_tiles):
                b0 = t * SAMPLES_PER_TILE
                for z1l in range(Z1L):
                    s = (t * Z1L + z1l) % 2
                    k = t * Z1L + z1l + 1
                    # RAW: both halves (DVE e=0, GpSimd e=1) done
                    scalar.wait_ge(cp_sem, k)
                    scalar.wait_ge(cp_gp_sem, k)
                    scalar.dma_start(
                        out=yv[b0:b0 + SAMPLES_PER_TILE, :, z1l],
                        in_=touts[s][:],
                    ).then_inc(st_sems[s], 16)

    return nc


_NC_CACHE: dict = {}


def _get_nc():
    if "nc" not in _NC_CACHE:
        _NC_CACHE["nc"] = build_nc()
    return _NC_CACHE["nc"]


def kernel(data: np.ndarray, _trace: bool = False):
    data = np.ascontiguousarray(data, dtype=np.float32)
    assert data.shape == (B, 2, 65536), data.shape
    nc = _get_nc()
    in_maps = [{"x": data[i * B_PER_CORE:(i + 1) * B_PER_CORE]}
               for i in range(N_CORES)]
    res = run_bass_kernel_spmd(nc, in_maps, list(range(N_CORES)),
                               trace=_trace)
    out = np.concatenate([res.results[i]["y"] for i in range(N_CORES)], axis=0)
    if _trace:
        return out, res
    return out


# revision 20
# speedup vs baseline: 1.1404x; 1.1404x over previous
"""Trainium2 Bass kernel for nn_DataPreprocessor: row-interleave + 16x16 patch
extraction, implemented as a pure data-movement (permutation) kernel.

Reference semantics (per sample):
  data: [2, 65536] -> R: [256, 512] with R[2k]=data[0].reshape(128,512)[k],
  R[2k+1]=data[1].reshape(128,512)[k] -> non-overlapping 16x16 patches,
  row-major, each flattened -> out: [512, 256].

Index algebra (per sample), with z1 in [0,16), z2 in [0,32), ph in [0,8),
e in [0,2), q in [0,16):
  out[z1*32+z2, (2*ph+e)*16+q] = data[e, z1*4096 + ph*512 + z2*16 + q]
With out flat = z1*8192 + z2*256 + ph*32 + e*16 + q the kernel is the pure
5D transpose (e, z1, ph, z2, q) -> (z1, z2, ph, e, q).

Strategy: batch-shard 256 samples over 8 cores (32/core), processed as 2
tiles of 16 samples. Split z1 = z1h*2 + z1l; SBUF partition p = b*8 + z1h
(b in [0,16) local). Then:
  - loads (one per e): HBM AP [b:16][z1h:8][(z1l r):8192] -- 32KB contiguous
    descriptors, outer dim 16 so the HWDGE spreads over all 16 SDMA engines
  - shuffle: 4 DVE copies per tile, (e,z1l)-indexed, permuting the free dim
    (ph,z2,q) -> (z2,ph,q) blocks into out order within each partition
  - stores (one per z1l): HBM AP [b:16][z1h:8][8192] -- 32KB descriptors,
    outer 16
All DMA descriptors are the full-rate 32KB shape (the 181us -> 117us -> now
journey: never give HWDGE an AP whose outer dim is <16, and never let
descriptors shrink below 32KB when avoidable).

Raw bass (not Tile): walrus's DMA_DIRECT2D struct admits only one sync-wait
command per DMA, so cross-engine ordering uses standalone wait_ge
instructions. DMA-completion semaphores arrive as 16 independent +1s per
DMA, so each wait threshold must only ever count DMAs covered by it:
dedicated sems per (tile, e) for loads and per tout-slot for stores.
"""

import sys

for _p in ("/opt/trn_rl_repo",):
    if _p not in sys.path:
        sys.path.insert(0, _p)

import numpy as np

import concourse.bass as bass
import concourse.mybir as mybir
from concourse.bass_utils import run_bass_kernel_spmd

N_CORES = 8
B = 256
B_PER_CORE = B // N_CORES          # 32
SAMPLES_PER_TILE = 16              # 16 samples x 8 z1h = 128 partitions
Z1H, Z1L, Z2, PH, E, QQ = 8, 2, 32, 8, 2, 16
FREE_IN = E * Z1L * PH * Z2 * QQ   # 16384 elements = 64KB per partition
FREE_OUT = PH * Z2 * E * QQ        # 8192 elements = 32KB per partition
NPART = 128


def build_nc(b_per_core: int = B_PER_CORE) -> bass.Bass:
    n_tiles = b_per_core // SAMPLES_PER_TILE
    f32 = mybir.dt.float32

    nc = bass.Bass()
    x = nc.dram_tensor("x", [b_per_core, 2, 65536], f32, kind="ExternalInput")
    y = nc.dram_tensor("y", [b_per_core, 512, 256], f32,
                       kind="ExternalOutput")

    # load view: [b, z1h, e, z1l, r] ; r is a 16KB contiguous run
    xv = x.rearrange("b e (z1h z1l r) -> b z1h e z1l r", z1h=Z1H, z1l=Z1L)
    # store view: [b, z1h, z1l, (z2 c)] ; (z2 c) is a 32KB contiguous run
    yv = y.rearrange("b (z1h z1l z2) c -> b z1h z1l (z2 c)",
                     z1h=Z1H, z1l=Z1L)

    with (
        nc.sbuf_tensor([NPART, FREE_IN], f32) as tin0,
        nc.sbuf_tensor([NPART, FREE_IN], f32) as tin1,
        nc.sbuf_tensor([NPART, FREE_OUT], f32) as tout0,
        nc.sbuf_tensor([NPART, FREE_OUT], f32) as tout1,
        nc.semaphore("ld00") as ld00,
        nc.semaphore("ld01") as ld01,
        nc.semaphore("ld10") as ld10,
        nc.semaphore("ld11") as ld11,
        nc.semaphore("st0") as st0,
        nc.semaphore("st1") as st1,
        nc.semaphore("cp_sem") as cp_sem,
        nc.Block() as block,
    ):
        tins = [tin0, tin1]
        touts = [tout0, tout1]
        # one sem per (tile, z1l) load-quarter-pair: copy (t,z1l,e) needs
        # only the (e,z1l) quarter, so keying sems by z1l lets the first
        # store start after 2 quarter-loads instead of 4
        ld_sems = [[ld00, ld01], [ld10, ld11]]
        st_sems = [st0, st1]

        @block.sync
        def _(sync):
            # loads stream back-to-back with no waits: each tile has its
            # own tin buffer, so there is no SBUF reuse hazard on loads.
            # One DMA per (e, z1l) quarter: 16KB descriptors -- measured
            # faster on the HBM read side than 32KB descriptors that comb
            # over the e-interleave at 50% duty. e-major issue order keeps
            # consecutive DMAs reading adjacent HBM regions.
            for t in range(n_tiles):
                b0 = t * SAMPLES_PER_TILE
                for e in range(E):
                    for z1l in range(Z1L):
                        off = e * 8192 + z1l * 4096
                        sync.dma_start(
                            out=tins[t][:, off:off + 4096],
                            in_=xv[b0:b0 + SAMPLES_PER_TILE, :, e, z1l],
                        ).then_inc(ld_sems[t][z1l], 16)

        @block.vector
        def _(vector):
            for t in range(n_tiles):
                tin = tins[t]
                for z1l in range(Z1L):
                    s = (t * Z1L + z1l) % 2
                    tout = touts[s]
                    if t * Z1L + z1l >= 2:
                        # WAR: the store that last read this tout slot
                        vector.wait_ge(st_sems[s], 16 * ((t * Z1L + z1l) // 2))
                    # both e quarter-loads of this (tile, z1l)
                    vector.wait_ge(ld_sems[t][z1l], 32)
                    for e in range(E):
                        # src: f = e*8192 + z1l*4096 + ph*512 + z2*16 + q
                        src = tin.rearrange(
                            "p (e z1l ph z2 q) -> p e z1l ph z2 q",
                            e=E, z1l=Z1L, ph=PH, z2=Z2, q=QQ)[:, e, z1l]
                        # dst: f' = z2*256 + ph*32 + e*16 + q
                        dst = tout.rearrange(
                            "p (z2 ph e q) -> p e ph z2 q",
                            z2=Z2, ph=PH, e=E, q=QQ)[:, e]
                        vector.tensor_copy(dst, src).then_inc(cp_sem, 1)

        @block.scalar
        def _(scalar):
            for t in range(n_tiles):
                b0 = t * SAMPLES_PER_TILE
                for z1l in range(Z1L):
                    s = (t * Z1L + z1l) % 2
                    # RAW: both copies (e=0,1) for this (t, z1l) done
                    scalar.wait_ge(cp_sem, 4 * t + 2 * z1l + 2)
                    scalar.dma_start(
                        out=yv[b0:b0 + SAMPLES_PER_TILE, :, z1l],
                        in_=touts[s][:],
                    ).then_inc(st_sems[s], 16)

    return nc


_NC_CACHE: dict = {}


def _get_nc():
    if "nc" not in _NC_CACHE:
        _NC_CACHE["nc"] = build_nc()
    return _NC_CACHE["nc"]


def kernel(data: np.ndarray, _trace: bool = False):
    data = np.ascontiguousarray(data, dtype=np.float32)
    assert data.shape == (B, 2, 65536), data.shape
    nc = _get_nc()
    in_maps = [{"x": data[i * B_PER_CORE:(i + 1) * B_PER_CORE]}
               for i in range(N_CORES)]
    res = run_bass_kernel_spmd(nc, in_maps, list(range(N_CORES)),
                               trace=_trace)
    out = np.concatenate([res.results[i]["y"] for i in range(N_CORES)], axis=0)
    if _trace:
        return out, res
    return out
